# revision 28
# baseline (speedup 1.0000x reference)
"""Trainium2 Bass kernel for StyleGAN2-style upsampled Conv1d.

Reference computation (for x:(16,256,4096), weight:(256,256,3), bias:(256,)):
  y = conv_transpose1d(x, weight, stride=2)      # correlation on 2x-dilated x
  z = upfirdn1d(y, [1,3,3,1]/8 * 2)              # depthwise FIR
  out = z + bias                                  # (16, 256, 8192)

The transposed conv + FIR collapse into TWO 3-tap correlations over the
original x grid (even/odd output phases):
  out[:, :, 2j]   = A @x[j-1] + B @x[j]  + C @x[j+1]
  out[:, :, 2j+1] = A'@x[j-1] + B'@x[j]  + C'@x[j+1]
with (w0,w1,w2) = weight taps:
  A  = .75 w0 + .25 w1   B  = .25 w0 + .75 w1 + .75 w2   C  = .25 w2
  A' = .25 w0            B' = .75 w0 + .75 w1 + .25 w2   C' = .25 w1 + .75 w2

On-chip: chunk-major streaming — for each NCHUNK-position chunk, 12
accumulating bf16 matmuls (2 phases x 3 taps x 2 K-tiles) fill one
PSUM pair, which drains immediately (bias add + even/odd interleave in
one vector or scalar op) and DMAs out bf16 (host upcasts to fp32).
bf16 matters twice: matmul issue period drops from 570 to ~518 cycles
(fp32 LDWEIGHTS/SBUF pressure gone) and every DMA byte halves.  The
stream then runs at the PE floor (~216 ns per 512-row matmul, 2.4 GHz).
Sharding: data-parallel over batch (2 per core x 8 cores).

Timeline model (measured): ~7us fixed framework prologue; every DMA
queue has ~2.5us issue-to-data latency; SWDGE ~390 GB/s, HWDGE queues
~190 GB/s each, ~400-450 GB/s shared.  Critical first tiles ride two
queues in parallel (x heads on sync HWDGE, m=0 weights on SWDGE with
the first 2 blocks as their own small DMA) so the real stream starts
~10.4-11us; NWARM dummy matmuls bridge the gap
from the prologue to first-data and run the HAM clock ramp.  The last
512-chunk is split in two so the final drain+store tail is short; the
final drain is a single vector op (a split drain's scalar half gets
its wait threshold merged past the tail dummies and starts ~0.7us
late), and store issues round-robin over three queues.

Not under kernel control (measured): ~1.5us of stalls at an exact
10.8us period (external HW event, survives any restructuring), ~2.5us
DMA issue-to-data latency, the ~7us prologue, and occasional whole-run
1.2x slowdowns when the shared device drops to 2.0 GHz.
"""

import numpy as np

import concourse.bass as bass
import concourse.mybir as mybir
import concourse.tile as tile
from concourse import bacc
from concourse.bass_utils import run_bass_kernel_spmd

N, IN_CH, OUT_CH, KERNEL, D = 16, 256, 256, 3, 4096
NCORES = 8
BPC = N // NCORES          # batches per core
DOUT = 2 * D
F32 = mybir.dt.float32
F32R = mybir.dt.float32r

BF16 = mybir.dt.bfloat16

NCHUNK = 512               # matmul moving free dim (= one PSUM bank of fp32)
NCHUNKS = D // NCHUNK
HEAD = NCHUNK + 3          # x columns needed by chunk 0
NWARM = 7                   # leading dummy matmuls: clock ramp while DMAs land
NTAIL = 8                  # trailing dummies: hold the clock through epilogue

_CACHED = {}


def _wblk(phase, tap, k, m):
    # m-major so each m-half of the weights is one contiguous DMA
    return ((m * 2 + phase) * 3 + tap) * 2 + k


def _build_nc(mm_dtype=BF16):
    nc = bacc.Bacc("TRN2", target_bir_lowering=False, debug=False)

    # x arrives host-padded with zero columns at 0 and D+1 (3-tap halo),
    # pre-cast to bf16 (tolerance is 2e-2; bf16 rounding costs ~4e-3).
    # bf16 halves every DMA byte, halves LDWEIGHTS/SBUF traffic, and the
    # PE still accumulates in fp32 PSUM at the same 1 row/cycle.
    x_t = nc.dram_tensor("x", [BPC, IN_CH, D + 2], mm_dtype, kind="ExternalInput")
    # w layout: 24 blocks of (128 K, 128 M); see _wblk
    w_t = nc.dram_tensor("w", [128, 24 * 128], mm_dtype, kind="ExternalInput")
    b_t = nc.dram_tensor("b", [128, 2], F32, kind="ExternalInput")
    # Output leaves the chip as bf16 (half the store bytes); the host
    # upcasts back to fp32.
    o_t = nc.dram_tensor("out", [BPC, OUT_CH, DOUT], BF16, kind="ExternalOutput")

    pbufs = 8 * 512 // (2 * NCHUNK)   # PSUM pairs that fit in 8 banks
    with tile.TileContext(nc) as tc:
        with (
            tc.tile_pool(name="wpool", bufs=1) as wpool,
            tc.tile_pool(name="xpool", bufs=2 * BPC) as xpool,
            tc.tile_pool(name="zpool", bufs=2 * pbufs) as zpool,
            tc.tile_pool(name="ppool", bufs=pbufs, space="PSUM") as ppool,
        ):
            w_sb = wpool.tile([128, 24 * 128], mm_dtype)
            b_sb = wpool.tile([128, 2], F32)
            x_sb = {}
            for bb in range(BPC):
                for k in range(2):
                    x_sb[bb, k] = xpool.tile(
                        [128, D + 2], mm_dtype, tag="x", name=f"x_{bb}_{k}"
                    )

            # Early DMA bandwidth is one shared ~400 GB/s pool with a
            # ~2.5us issue-to-data lag, so what matters is issue ORDER and
            # keeping the two first-use tiles (x chunk-0 heads, m=0
            # weights) on separate queues; everything else queues behind.
            warm_bf = wpool.tile([128, 128 + 512], mybir.dt.bfloat16)
            nc.vector.memset(warm_bf[:], 1.0)

            nc.scalar.dma_start(out=b_sb[:], in_=b_t[:])
            # critical first tiles ride two queues in parallel: x chunk-0
            # heads on the sync HWDGE (whose queue is idle until the first
            # store ~13.5us), m=0 weights on the SWDGE
            for k in range(2):
                nc.sync.dma_start(
                    out=x_sb[0, k][:, 0:HEAD],
                    in_=x_t[0, k * 128:(k + 1) * 128, 0:HEAD],
                )
            # first matmul needs only block 0-1 (64KB): land it first so
            # the weight gate moves ~0.5us earlier than one 196KB DMA
            nc.gpsimd.dma_start(out=w_sb[:, 0:256], in_=w_t[:, 0:256])
            nc.gpsimd.dma_start(out=w_sb[:, 256:768], in_=w_t[:, 256:768])
            nc.gpsimd.dma_start(out=w_sb[:, 768:1536], in_=w_t[:, 768:1536])
            # Few, growing cuts: every x DMA adds a semaphore-wait in the
            # Tensor queue at the first chunk that reads past its boundary,
            # and each such wait splits an LDWEIGHTS/matmul pair (~215ns
            # bubble).  4 cuts per bb=0 tile (sized so arrival beats the
            # stream) + whole-tile bb=1 keeps the wait count minimal.
            cuts = [HEAD, 1539, 2563, D + 2]
            for t in range(len(cuts) - 1):
                for k in range(2):
                    lo, hi = cuts[t], cuts[t + 1]
                    nc.gpsimd.dma_start(
                        out=x_sb[0, k][:, lo:hi],
                        in_=x_t[0, k * 128:(k + 1) * 128, lo:hi],
                    )
            nc.gpsimd.dma_start(out=w_sb[:, 1536:3072], in_=w_t[:, 1536:3072])
            for k in range(2):
                nc.gpsimd.dma_start(
                    out=x_sb[1, k][:], in_=x_t[1, k * 128:(k + 1) * 128, :]
                )

            # PE warmup while the first DMAs land: dummy bf16 matmuls on
            # a memset tile run the HAM clock-gate ramp (~3us + margin)
            # so the real stream starts at the full 2.4 GHz.  The PSUM
            # garbage lands in a pool slot that a later real
            # accumulation group's start=True clears.
            warm_ps = ppool.tile([128, 2 * NCHUNK], F32, tag="pair", name="warm_ps")
            for _ in range(NWARM):
                nc.tensor.matmul(
                    warm_ps[:, 0:512],
                    lhsT=warm_bf[:, 0:128],
                    rhs=warm_bf[:, 128:640],
                    start=True,
                    stop=True,
                )

            # Chunk list: (bb, m, x0, width).  The very last 512-chunk is
            # split into two 256-halves so the final drain+store after the
            # last matmul covers 4x less data (256-row matmuls still beat
            # the ~97ns LDWEIGHTS floor, so the PE loses nothing).
            seq = []
            for bb in range(BPC):
                for m in range(2):
                    lastrow = bb == BPC - 1 and m == 1
                    for c in range(NCHUNKS):
                        if lastrow and c == NCHUNKS - 1:
                            seq.append((bb, m, c * NCHUNK, NCHUNK // 2))
                            seq.append((bb, m, c * NCHUNK + NCHUNK // 2,
                                        NCHUNK // 2))
                        else:
                            seq.append((bb, m, c * NCHUNK, NCHUNK))

            # Store issues round-robin over three queues (sync/scalar
            # HWDGE + gpsimd SWDGE) so no engine's issue chain delays the
            # final drains.
            for ci, (bb, m, x0, width) in enumerate(seq):
                pair = ppool.tile([128, 2 * width], F32, tag="pair",
                                  name=f"pair_{bb}_{m}_{x0}")
                for phase in range(2):
                    for tap in range(3):
                        for k in range(2):
                            w_ap = w_sb[:, _wblk(phase, tap, k, m) * 128:][:, :128]
                            rhs = x_sb[bb, k][:, x0 + tap:x0 + tap + width]
                            nc.tensor.matmul(
                                pair[:, phase * width:(phase + 1) * width],
                                lhsT=w_ap,
                                rhs=rhs,
                                start=(tap == 0 and k == 0),
                                stop=(tap == 2 and k == 1),
                            )
                bias_ap = b_sb[:, m:m + 1]
                zt = zpool.tile([128, 2 * width], BF16, tag="z",
                                name=f"z_{bb}_{m}_{x0}")
                # psum pair is [even(width) | odd(width)]; writing in
                # (phase, j) order at stride 2 interleaves the two
                # phases while adding bias.
                last = ci == len(seq) - 1
                vin = pair[:].rearrange("p (two j) -> p two j", two=2)
                if not last:
                    vout = zt[:].rearrange("p (j two) -> p two j", two=2)
                    if ci % 2 == 0:
                        nc.vector.tensor_scalar(
                            out=vout, in0=vin,
                            scalar1=bias_ap, scalar2=None,
                            op0=mybir.AluOpType.add,
                        )
                    else:
                        nc.scalar.activation(
                            out=vout, in_=vin,
                            func=mybir.ActivationFunctionType.Identity,
                            bias=bias_ap,
                        )
                    # keep scalar free of store issues near the end so the
                    # final half-drain on scalar starts the moment the last
                    # matmul retires
                    if ci >= len(seq) - 4:
                        oeng = (nc.sync, nc.gpsimd)[ci % 2]
                    else:
                        oeng = (nc.sync, nc.scalar, nc.gpsimd)[ci % 3]
                    oeng.dma_start(
                        out=o_t[bb, m * 128:(m + 1) * 128,
                                2 * x0:2 * x0 + 2 * width],
                        in_=zt[:],
                    )
                else:
                    # Final (quarter-size) chunk: ONE vector drain -- the
                    # scalar engine's drain wait threshold gets merged past
                    # the tail dummies, so splitting the drain across
                    # vector+scalar makes scalar start ~0.7us late.  The
                    # two stores still ride both HWDGE queues.
                    vout = zt[:].rearrange("p (j two) -> p two j", two=2)
                    nc.vector.tensor_scalar(
                        out=vout, in0=vin,
                        scalar1=bias_ap, scalar2=None,
                        op0=mybir.AluOpType.add,
                    )
                    for h, dma_eng in enumerate((nc.sync, nc.scalar)):
                        dma_eng.dma_start(
                            out=o_t[bb, m * 128:(m + 1) * 128,
                                    2 * x0 + h * width:
                                    2 * x0 + (h + 1) * width],
                            in_=zt[:, h * width:(h + 1) * width],
                        )

            # Trailing dummies keep the PE active (and the clock gate at
            # 8/8) while the final drains + stores retire, so the
            # framework's epilogue barrier doesn't run at 1/2 clock.
            tail_ps = ppool.tile([128, 2 * NCHUNK], F32, tag="pair",
                                 name="tail_ps")
            for _ in range(NTAIL):
                nc.tensor.matmul(
                    tail_ps[:, 0:512],
                    lhsT=warm_bf[:, 0:128],
                    rhs=warm_bf[:, 128:640],
                    start=True,
                    stop=True,
                )
    nc.compile()
    return nc


def _host_weights(weight, bias):
    w = np.asarray(weight, dtype=np.float32)
    w0, w1, w2 = w[:, :, 0], w[:, :, 1], w[:, :, 2]
    taps = [
        [0.75 * w0 + 0.25 * w1, 0.25 * w0 + 0.75 * w1 + 0.75 * w2, 0.25 * w2],
        [0.25 * w0, 0.75 * w0 + 0.75 * w1 + 0.25 * w2, 0.25 * w1 + 0.75 * w2],
    ]
    w_host = np.zeros((128, 24 * 128), dtype=np.float32)
    for phase in range(2):
        for tap in range(3):
            for k in range(2):
                for m in range(2):
                    blk = _wblk(phase, tap, k, m)
                    # lhsT block[i, o] = W[phase][tap][m*128+o, k*128+i]
                    wt = taps[phase][tap][m * 128:(m + 1) * 128, k * 128:(k + 1) * 128]
                    w_host[:, blk * 128:(blk + 1) * 128] = wt.T
    b_host = np.asarray(bias, dtype=np.float32).reshape(2, 128).T.copy()
    import ml_dtypes
    return w_host.astype(ml_dtypes.bfloat16), b_host


def _host_x(x):
    import ml_dtypes
    x = np.asarray(x, dtype=np.float32)
    xp = np.pad(x, ((0, 0), (0, 0), (1, 1)))
    return np.ascontiguousarray(xp.astype(ml_dtypes.bfloat16))


def _host_out(results):
    return np.concatenate(
        [np.asarray(r["out"]).astype(np.float32) for r in results], axis=0
    )


def kernel(x, weight, bias):
    x = _host_x(x)
    w_host, b_host = _host_weights(weight, bias)

    if "nc" not in _CACHED:
        _CACHED["nc"] = _build_nc()
    nc = _CACHED["nc"]

    in_maps = []
    for core in range(NCORES):
        shard = np.ascontiguousarray(x[core * BPC:(core + 1) * BPC])
        in_maps.append({"x": shard, "w": w_host, "b": b_host})

    res = run_bass_kernel_spmd(nc, in_maps, core_ids=list(range(NCORES)))
    return _host_out(res.results)

